# revision 1
# baseline (speedup 1.0000x reference)
"""ContrastiveCenterLoss on 8 Trainium2 NeuronCores.

Math: with dist[b,c] = ||f_b - c_c||^2,
  intra = sum_b dist[b, label_b]          = sum_b ||f_b - c_{label_b}||^2
  total = sum_{b,c} dist[b,c]             = C*sum||f||^2 + B*sum||c||^2 - 2*(sum_b f_b)@(sum_c c_c)
  inter = total - intra
  loss  = (1/2/B) * intra / (inter + 1e-6) / 0.1

Sharding: feat/label batch-sharded (2048 rows/core); centers statistics
sharded over 512-row slices; the full centers table stays in HBM and is
row-gathered by label via indirect DMA. Host all-reduces the per-core
partial sums in float64 and applies the final scalar division.
"""

import numpy as np

B, C, D = 16384, 4096, 128
LAMBDA_C = 1.0
NCORES = 8
BS = B // NCORES          # 2048 feat rows per core
NPT = BS // 128           # 16 feat rows per partition
NCHUNK = 4                # feat processed in 4 chunks of 512 free-dim cols
CPC = NPT // NCHUNK       # 4 row-blocks per chunk
CS = C // NCORES          # 512 center rows per core (stats slice)
CSPT = CS // 128          # 4 center rows per partition

_cached = {}


def _build_nc(repeat=1, gather_mode="indirect"):
    import concourse.bass as bass
    import concourse.tile as tile
    from concourse import bacc, mybir

    f32 = mybir.dt.float32
    i32 = mybir.dt.int32

    nc = bacc.Bacc("TRN2", target_bir_lowering=False, debug=False,
                   num_devices=NCORES)

    feat = nc.dram_tensor("feat", [BS, D], f32, kind="ExternalInput")
    labt = nc.dram_tensor("labt", [128, NPT], i32, kind="ExternalInput")
    centers = nc.dram_tensor("centers", [C, D], f32, kind="ExternalInput")
    cslice = nc.dram_tensor("cslice", [CS, D], f32, kind="ExternalInput")

    o_fsq = nc.dram_tensor("o_fsq", [128, NCHUNK], f32, kind="ExternalOutput")
    o_intra = nc.dram_tensor("o_intra", [128, NCHUNK], f32, kind="ExternalOutput")
    o_csq = nc.dram_tensor("o_csq", [128, 1], f32, kind="ExternalOutput")
    o_vec = nc.dram_tensor("o_vec", [1, 1024], f32, kind="ExternalOutput")

    CW = CPC * D  # 512 free-dim columns per chunk

    with tile.TileContext(nc) as tc:
        with tc.tile_pool(name="const", bufs=1) as cpool, \
             tc.tile_pool(name="sbuf", bufs=2) as pool, \
             tc.tile_pool(name="scratch", bufs=2) as spool, \
             tc.tile_pool(name="psum", bufs=2, space="PSUM") as psum:

            ones = cpool.tile([128, 1], f32)
            nc.vector.memset(ones[:], 1.0)

            # partition p holds feat rows p*NPT .. p*NPT+NPT-1 (contiguous 8KB)
            fv = feat.ap().rearrange("(p n) d -> p n d", p=128)
            csv = cslice.ap().rearrange("(p n) d -> p n d", p=128)

            for _ in range(repeat):
                # indices first so gathers can start early
                lab = pool.tile([128, NPT], i32, tag="lab")
                nc.sync.dma_start(out=lab[:], in_=labt.ap())

                o_fsq_t = pool.tile([128, NCHUNK], f32, tag="o_fsq_t")
                o_intra_t = pool.tile([128, NCHUNK], f32, tag="o_intra_t")
                o_csq_t = pool.tile([128, 1], f32, tag="o_csq_t")
                vec_sb = pool.tile([1, 1024], f32, tag="vec_sb")

                ps_f = psum.tile([1, CW], f32, tag="ps_f")
                ps_c = psum.tile([1, CW], f32, tag="ps_c")

                # centers-slice statistics (independent of feat path)
                cs_t = pool.tile([128, CSPT * D], f32, tag="cs_t")
                nc.sync.dma_start(out=cs_t[:], in_=csv[:, :, :])
                cs_scr = pool.tile([128, CSPT * D], f32, tag="cs_scr")
                nc.scalar.activation(out=cs_scr[:], in_=cs_t[:],
                                     func=mybir.ActivationFunctionType.Square,
                                     accum_out=o_csq_t[:, 0:1])
                nc.tensor.matmul(out=ps_c[:], lhsT=ones[:], rhs=cs_t[:],
                                 start=True, stop=True)

                for k in range(NCHUNK):
                    f_c = spool.tile([128, CW], f32, tag="f_c")
                    nc.sync.dma_start(out=f_c[:],
                                      in_=fv[:, k * CPC:(k + 1) * CPC, :])
                    cg_c = spool.tile([128, CW], f32, tag="cg_c")
                    if gather_mode == "indirect":
                        for j in range(CPC):
                            nc.gpsimd.indirect_dma_start(
                                out=cg_c[:, j * D:(j + 1) * D],
                                out_offset=None,
                                in_=centers.ap(),
                                in_offset=bass.IndirectOffsetOnAxis(
                                    ap=lab[:, k * CPC + j:k * CPC + j + 1],
                                    axis=0),
                            )
                    else:  # "fake": plain DMA of same volume (timing expt)
                        cv = centers.ap().rearrange(
                            "(q p n) d -> q p n d", p=128, n=CPC)
                        nc.sync.dma_start(out=cg_c[:], in_=cv[k])
                    # sum of f^2 on ACT
                    f_scr = spool.tile([128, CW], f32, tag="f_scr")
                    nc.scalar.activation(
                        out=f_scr[:], in_=f_c[:],
                        func=mybir.ActivationFunctionType.Square,
                        accum_out=o_fsq_t[:, k:k + 1])
                    # column sums of f on PE (accumulated over chunks)
                    nc.tensor.matmul(out=ps_f[:], lhsT=ones[:], rhs=f_c[:],
                                     start=(k == 0), stop=(k == NCHUNK - 1))
                    # intra partial on DVE: d = f - cg; accum += d*d
                    d_c = spool.tile([128, CW], f32, tag="d_c")
                    nc.vector.tensor_sub(d_c[:], f_c[:], cg_c[:])
                    d_scr = spool.tile([128, CW], f32, tag="d_scr")
                    nc.vector.scalar_tensor_tensor(
                        out=d_scr[:], in0=d_c[:], scalar=1.0, in1=d_c[:],
                        op0=mybir.AluOpType.mult, op1=mybir.AluOpType.mult,
                        accum_out=o_intra_t[:, k:k + 1])

                nc.vector.tensor_copy(vec_sb[:, 0:CW], ps_f[:])
                nc.scalar.copy(vec_sb[:, CW:2 * CW], ps_c[:])

                nc.sync.dma_start(out=o_fsq.ap(), in_=o_fsq_t[:])
                nc.sync.dma_start(out=o_intra.ap(), in_=o_intra_t[:])
                nc.sync.dma_start(out=o_csq.ap(), in_=o_csq_t[:])
                nc.sync.dma_start(out=o_vec.ap(), in_=vec_sb[:])

    nc.compile()
    return nc


def _get_nc(repeat=1, gather_mode="indirect"):
    key = ("nc", repeat, gather_mode)
    if key not in _cached:
        _cached[key] = _build_nc(repeat, gather_mode)
    return _cached[key]


def _make_in_maps(feat, label, centers):
    feat = np.ascontiguousarray(np.asarray(feat, dtype=np.float32))
    centers = np.ascontiguousarray(np.asarray(centers, dtype=np.float32))
    lab = np.asarray(label).astype(np.int32)
    in_maps = []
    for k in range(NCORES):
        fs = feat[k * BS:(k + 1) * BS]
        ls = lab[k * BS:(k + 1) * BS].reshape(128, NPT)
        cs = centers[k * CS:(k + 1) * CS]
        in_maps.append({
            "feat": np.ascontiguousarray(fs),
            "labt": np.ascontiguousarray(ls),
            "centers": centers,
            "cslice": np.ascontiguousarray(cs),
        })
    return in_maps


def _combine(results):
    sum_fsq = 0.0
    intra = 0.0
    sum_csq = 0.0
    F = np.zeros(D, dtype=np.float64)
    Cv = np.zeros(D, dtype=np.float64)
    for r in results:
        sum_fsq += r["o_fsq"].astype(np.float64).sum()
        intra += r["o_intra"].astype(np.float64).sum()
        sum_csq += r["o_csq"].astype(np.float64).sum()
        v = r["o_vec"][0].astype(np.float64)
        F += v[:512].reshape(4, 128).sum(axis=0)
        Cv += v[512:].reshape(4, 128).sum(axis=0)
    total = C * sum_fsq + B * sum_csq - 2.0 * float(F @ Cv)
    inter = total - intra
    loss = (LAMBDA_C / 2.0 / B) * intra / (inter + 1e-6) / 0.1
    return np.float32(loss)


def kernel(feat, label, centers):
    from concourse.bass_utils import run_bass_kernel_spmd

    nc = _get_nc()
    in_maps = _make_in_maps(feat, label, centers)
    res = run_bass_kernel_spmd(nc, in_maps, list(range(NCORES)))
    return _combine(res.results)



# revision 35
# speedup vs baseline: 1.9477x; 1.9477x over previous
"""ContrastiveCenterLoss on 8 Trainium2 NeuronCores.

Math: with dist[b,c] = ||f_b - c_c||^2,
  intra = sum_b ||f_b||^2 + sum_b ||c_{lab_b}||^2 - 2*sum_b f_b.c_{lab_b}
  total = sum_{b,c} dist[b,c] = C*sum||f||^2 + B*sum||c||^2 - 2*(sum_b f_b)@(sum_c c_c)
  inter = total - intra
  loss  = (1/2/B) * intra / (inter + 1e-6) / 0.1

Sharding: feat/label batch-sharded (2048 rows/core); centers statistics
sharded over 512-row slices; the full centers table stays in HBM and is
row-gathered by label via batched indirect DMA. Host all-reduces the
per-core partial sums in float64 and applies the final scalar division.
"""

import numpy as np

B, C, D = 16384, 4096, 128
LAMBDA_C = 1.0
NCORES = 8
BS = B // NCORES          # 2048 feat rows per core
NPT = BS // 128           # 16 feat rows per partition
CS = C // NCORES          # 512 center rows per core (stats slice)
CSPT = CS // 128          # 4 center rows per partition

# schedule knobs (tuned via TimelineSim sweep)
CFG = dict(
    gsplit=(6, 6, 4),      # label-cols (NPT units) per indirect-DMA call
    fsplit=(4, 4, 4, 4),   # label-cols per feat dma_start
    mode="dot",            # cce: gather computes d=c-f via DMA compute op,
                           #      one square-accum per chunk gives intra
                           # dot: separate sum(c^2) and sum(f.c) accums
    dsq_eng=("act", "dve", "act"),         # cce: engine per gather chunk
    dcopy_eng=("dve", "dve", "dve"),       # cce: f->dest copy engine
    csq_eng=("act", "act", "act"),  # dot: per gather chunk: act|dve
    fsq_eng=("dve", "dve", "act", "act"),  # per fsplit chunk: act|dve
    cs_dma="pool",         # centers-slice load path: sp|pool
    vec_out="split",       # ps_f/ps_c egress: split|copy
    oacc_dma="scatter",    # o_acc egress: sp|pool|scatter (SWDGE prep+trigger)
)

_cached = {}


def _build_nc(cfg=None):
    import concourse.bass as bass
    import concourse.tile as tile
    from concourse import bacc, mybir

    cfg = dict(CFG, **(cfg or {}))
    gsplit = cfg["gsplit"]
    fsplit = cfg["fsplit"]
    assert sum(gsplit) == NPT and sum(fsplit) == NPT

    f32 = mybir.dt.float32
    f32r = mybir.dt.float32r
    i32 = mybir.dt.int32

    nc = bacc.Bacc("TRN2", target_bir_lowering=False, debug=False,
                   num_devices=NCORES)

    feat = nc.dram_tensor("feat", [BS, D], f32, kind="ExternalInput")
    onest = nc.dram_tensor("onest", [128, 1], f32, kind="ExternalInput")
    labt = nc.dram_tensor("labt", [128, NPT], i32, kind="ExternalInput")
    centers = nc.dram_tensor("centers", [C, D], f32, kind="ExternalInput")
    cslice = nc.dram_tensor("cslice", [CS, D], f32, kind="ExternalInput")

    i16 = mybir.dt.int16

    NG = len(gsplit)
    NF = len(fsplit)
    # accum cols: fsq[0:NF], csqg[NF:NF+NG], fdotc[NF+NG:NF+2NG], csq last;
    # padded to 64 cols (256B rows) for the scatter-add egress path
    NCOL = NF + 2 * NG + 1
    OW = 16               # scattered cols; DRAM row stays 64 f32 (256B stride)
    assert NCOL <= OW
    o_acc = nc.dram_tensor("o_acc", [128, 64], f32, kind="ExternalOutput")
    o_vec = nc.dram_tensor("o_vec", [1, 1024], f32, kind="ExternalOutput")

    with tile.TileContext(nc) as tc:
        with tc.tile_pool(name="const", bufs=1) as cpool, \
             tc.tile_pool(name="sbuf", bufs=1) as pool, \
             tc.tile_pool(name="scratch", bufs=2) as spool, \
             tc.tile_pool(name="psum", bufs=1, space="PSUM") as psum:

            ones = cpool.tile([128, 1], f32r)
            ones_r = ones[:]

            # partition p holds feat rows p*NPT .. p*NPT+NPT-1 (contiguous 8KB)
            fv = feat.ap().rearrange("(p n) d -> p n d", p=128)
            csv = cslice.ap().rearrange("(p n) d -> p n d", p=128)

            # label indices first so gather desc-gen can start earliest
            lab = pool.tile([128, NPT], i32, tag="lab")
            nc.sync.dma_start(out=lab[:], in_=labt.ap())

            # centers slice (optionally on the otherwise-idle Pool SWDGE
            # path, freeing an HWDGE slot for the feat loads)
            cs_t = pool.tile([128, CSPT * D], f32, tag="cs_t")
            # f32r-typed DMAs (bit-identical to f32): the BIR verifier
            # requires producers feeding fp32r matmuls to be typed f32r
            if cfg["cs_dma"] == "pool":
                nc.gpsimd.dma_start(out=cs_t[:].bitcast(f32r),
                                    in_=csv[:, :, :].bitcast(f32r))
            else:
                nc.sync.dma_start(out=cs_t[:].bitcast(f32r),
                                  in_=csv[:, :, :].bitcast(f32r))
            # ones for the column-sum matmuls: DVE memset cannot write f32r
            # (ISA check), so load from a DRAM constant via f32r-typed DMA
            nc.gpsimd.dma_start(out=ones_r, in_=onest.ap().bitcast(f32r))

            o_t = pool.tile([128, OW], f32, tag="o_t")
            if cfg["oacc_dma"] == "scatter":
                nc.vector.memset(o_t[:], 0.0)

            # feat chunks
            f_all = pool.tile([128, NPT * D], f32, tag="f_all")
            off = 0
            for w in fsplit:
                nc.sync.dma_start(
                    out=f_all[:, off * D:(off + w) * D].bitcast(f32r),
                    in_=fv[:, off:off + w, :].bitcast(f32r))
                off += w

            ps_f = psum.tile([1, 512], f32, tag="ps_f")
            ps_c = psum.tile([1, 512], f32, tag="ps_c")

            # centers-slice statistics (independent of feat/gather path)
            cs_scr = spool.tile([128, CSPT * D], f32, tag="cs_scr")
            nc.scalar.activation(out=cs_scr[:], in_=cs_t[:],
                                 func=mybir.ActivationFunctionType.Square,
                                 accum_out=o_t[:, NCOL - 1:NCOL])
            nc.tensor.matmul(out=ps_c[:], lhsT=ones_r,
                             rhs=cs_t[:].bitcast(f32r),
                             start=True, stop=True)

            # feat column sums on PE (f32r: 1 cycle/row), fixed 512-wide
            # slices (PSUM bank limit), and squares on ACT per DMA chunk
            for k in range(4):
                sl = f_all[:, k * 512:(k + 1) * 512]
                nc.tensor.matmul(out=ps_f[:], lhsT=ones_r,
                                 rhs=sl.bitcast(f32r),
                                 start=(k == 0), stop=(k == 3))
            off = 0
            for k, w in enumerate(fsplit):
                sl = f_all[:, off * D:(off + w) * D]
                f_scr = spool.tile([128, w * D], f32, tag="f_scr")
                if cfg["fsq_eng"][k] == "act":
                    nc.scalar.activation(
                        out=f_scr[:], in_=sl,
                        func=mybir.ActivationFunctionType.Square,
                        accum_out=o_t[:, k:k + 1])
                else:
                    # pin early in the DVE queue so the scheduler cannot
                    # slot these behind the gather-dependent accums
                    with tc.high_priority():
                        nc.vector.scalar_tensor_tensor(
                            out=f_scr[:], in0=sl, scalar=1.0, in1=sl,
                            op0=mybir.AluOpType.mult, op1=mybir.AluOpType.mult,
                            accum_out=o_t[:, k:k + 1])
                off += w

            # column-sum vectors out (PSUM cannot DMA to DRAM directly,
            # and GPSIMD cannot read PSUM): ps_c copied early on DVE; ps_f
            # folded 512->256 in ONE DVE add issued before the gather
            # accums (it lands in the dead-zone before the first gather sem)
            vec_sb = pool.tile([1, 1024], f32, tag="vec_sb")
            nc.vector.tensor_copy(vec_sb[:, 0:512], ps_c[:])
            nc.vector.tensor_copy(vec_sb[:, 512:1024], ps_f[:])

            # gathered-centers path: batched indirect DMA + per-chunk accums
            cg_all = pool.tile([128, NPT * D], f32, tag="cg_all")
            off = 0
            for g, w in enumerate(gsplit):
                cg = cg_all[:, off * D:(off + w) * D]
                fsl = f_all[:, off * D:(off + w) * D]
                if cfg["mode"] == "cce":
                    # preload dest with f (early), then gather computes
                    # d = c - f in the DMA compute engine; one square-accum
                    # per chunk yields the intra contribution directly
                    ce = cfg["dcopy_eng"][g]
                    if ce == "act":
                        nc.scalar.copy(cg, fsl)
                    elif ce == "pool":
                        nc.gpsimd.tensor_copy(cg, fsl)
                    else:
                        nc.vector.tensor_copy(cg, fsl)
                    nc.gpsimd.indirect_dma_start(
                        out=cg,
                        out_offset=None,
                        in_=centers.ap(),
                        in_offset=bass.IndirectOffsetOnAxis(
                            ap=lab[:, off:off + w], axis=0),
                        compute_op=mybir.AluOpType.subtract,
                    )
                    d_scr = spool.tile([128, w * D], f32, tag="d_scr")
                    if cfg["dsq_eng"][g] == "act":
                        nc.scalar.activation(
                            out=d_scr[:], in_=cg,
                            func=mybir.ActivationFunctionType.Square,
                            accum_out=o_t[:, NF + g:NF + g + 1])
                    else:
                        nc.vector.scalar_tensor_tensor(
                            out=d_scr[:], in0=cg, scalar=1.0, in1=cg,
                            op0=mybir.AluOpType.mult,
                            op1=mybir.AluOpType.mult,
                            accum_out=o_t[:, NF + g:NF + g + 1])
                    off += w
                    continue
                nc.gpsimd.indirect_dma_start(
                    out=cg,
                    out_offset=None,
                    in_=centers.ap(),
                    in_offset=bass.IndirectOffsetOnAxis(
                        ap=lab[:, off:off + w], axis=0),
                )
                if cfg["csq_eng"][g] == "act":
                    c_scr = spool.tile([128, w * D], f32, tag="c_scr")
                    nc.scalar.activation(
                        out=c_scr[:], in_=cg,
                        func=mybir.ActivationFunctionType.Square,
                        accum_out=o_t[:, NF + g:NF + g + 1])
                else:
                    c_scr = spool.tile([128, w * D], f32, tag="c_scr")
                    nc.vector.scalar_tensor_tensor(
                        out=c_scr[:], in0=cg, scalar=1.0, in1=cg,
                        op0=mybir.AluOpType.mult, op1=mybir.AluOpType.mult,
                        accum_out=o_t[:, NF + g:NF + g + 1])
                d_scr = spool.tile([128, w * D], f32, tag="d_scr")
                nc.vector.scalar_tensor_tensor(
                    out=d_scr[:], in0=fsl, scalar=1.0, in1=cg,
                    op0=mybir.AluOpType.mult, op1=mybir.AluOpType.mult,
                    accum_out=o_t[:, NF + NG + g:NF + NG + g + 1])
                off += w

            if cfg["oacc_dma"] == "scatter":
                # SWDGE prep + trigger egress: the trigger path skips the
                # HWDGE/DGE setup (~1.3us) off the critical tail. The zero
                # fill rides the Pool SWDGE queue so its desc-gen (and hence
                # its transfer) naturally falls behind the gather stream;
                # the iota and descriptor prep follow it (WAW order: zero
                # write precedes the scatter-add), all before the vec copies.
                zs = cpool.tile([128, OW], f32)
                nc.vector.memset(zs[:], 0.0)
                # de-prioritize so the scheduler keeps the zero fill, iota
                # and prep behind the gather stream
                p0 = tc.cur_priority
                tc.cur_priority = p0 + 100000
                nc.gpsimd.dma_start(out=o_acc.ap()[:, 0:OW], in_=zs[:])
                sidx = cpool.tile([128, 8], i16)
                nc.gpsimd.iota(sidx[:], pattern=[[16, 8]], base=0,
                               channel_multiplier=1)
                oacc_sem = nc.alloc_semaphore("oacc_dma")
                nc.gpsimd.dma_scatter_add(
                    o_acc.ap()[:, 0:OW], o_t[:].unsqueeze(1), sidx[:],
                    128, 128, OW, elem_step=64,
                    prepare_only=True, sem=oacc_sem)
                tc.cur_priority = p0

            nc.sync.dma_start(out=o_vec.ap(), in_=vec_sb[:])

            if cfg["oacc_dma"] == "scatter":
                nc.gpsimd.trigger_dma(count=None)
            elif cfg["oacc_dma"] == "pool":
                nc.gpsimd.dma_start(out=o_acc.ap(), in_=o_t[:])
            else:
                nc.sync.dma_start(out=o_acc.ap(), in_=o_t[:])

    if cfg["oacc_dma"] == "scatter":
        # The prep's explicit completion sem occupies the descriptor's single
        # sem_num slot, so the auto-assigned DMASW lane sem never increments;
        # the exit drain still waits on it. Repoint those orphaned waits at
        # the real completion sem (same +16 contract).
        fn = nc.m.functions[0]
        updated = set()
        oacc_id = None
        for blk in fn.blocks:
            for inst in blk.instructions:
                si = inst.sync_info
                if si is None:
                    continue
                for u in si.on_update:
                    updated.add(u.id)
                    if u.ant_name == "oacc_dma":
                        oacc_id = u.id
        assert oacc_id is not None
        for blk in fn.blocks:
            for inst in blk.instructions:
                si = inst.sync_info
                if si is None:
                    continue
                for w in si.on_wait:
                    if (w.ant_name and w.ant_name.startswith("DMASW")
                            and w.id not in updated):
                        w.id = oacc_id
                        w.ant_name = "oacc_dma"

    nc.compile()
    return nc


def _get_nc():
    if "nc" not in _cached:
        _cached["nc"] = _build_nc()
    return _cached["nc"]


def _make_in_maps(feat, label, centers):
    feat = np.ascontiguousarray(np.asarray(feat, dtype=np.float32))
    centers = np.ascontiguousarray(np.asarray(centers, dtype=np.float32))
    lab = np.asarray(label).astype(np.int32)
    in_maps = []
    for k in range(NCORES):
        fs = feat[k * BS:(k + 1) * BS]
        ls = lab[k * BS:(k + 1) * BS].reshape(128, NPT)
        cs = centers[k * CS:(k + 1) * CS]
        in_maps.append({
            "feat": np.ascontiguousarray(fs),
            "onest": np.ones((128, 1), dtype=np.float32),
            "labt": np.ascontiguousarray(ls),
            "centers": centers,
            "cslice": np.ascontiguousarray(cs),
        })
    return in_maps


def _combine(results):
    NF = len(CFG["fsplit"])
    NG = len(CFG["gsplit"])
    sum_fsq = 0.0
    sum_csqg = 0.0
    sum_fdotc = 0.0
    sum_csq = 0.0
    F = np.zeros(D, dtype=np.float64)
    Cv = np.zeros(D, dtype=np.float64)
    for r in results:
        acc = r["o_acc"].astype(np.float64)
        sum_fsq += acc[:, 0:NF].sum()
        sum_csqg += acc[:, NF:NF + NG].sum()
        sum_fdotc += acc[:, NF + NG:NF + 2 * NG].sum()
        sum_csq += acc[:, NF + 2 * NG].sum()
        v = r["o_vec"][0].astype(np.float64)
        Cv += v[:512].reshape(4, 128).sum(axis=0)
        F += v[512:].reshape(4, 128).sum(axis=0)
    if CFG["mode"] == "cce":
        intra = sum_csqg          # those columns hold sum (c - f)^2
    else:
        intra = sum_fsq + sum_csqg - 2.0 * sum_fdotc
    total = C * sum_fsq + B * sum_csq - 2.0 * float(F @ Cv)
    inter = total - intra
    loss = (LAMBDA_C / 2.0 / B) * intra / (inter + 1e-6) / 0.1
    return np.float32(loss)


def kernel(feat, label, centers):
    from concourse.bass_utils import run_bass_kernel_spmd

    nc = _get_nc()
    in_maps = _make_in_maps(feat, label, centers)
    res = run_bass_kernel_spmd(nc, in_maps, list(range(NCORES)))
    return _combine(res.results)


# revision 45
# speedup vs baseline: 2.0669x; 1.0612x over previous
"""ContrastiveCenterLoss on 8 Trainium2 NeuronCores.

Math: with dist[b,c] = ||f_b - c_c||^2,
  intra = sum_b ||f_b||^2 + sum_b ||c_{lab_b}||^2 - 2*sum_b f_b.c_{lab_b}
  total = sum_{b,c} dist[b,c] = C*sum||f||^2 + B*sum||c||^2 - 2*(sum_b f_b)@(sum_c c_c)
  inter = total - intra
  loss  = (1/2/B) * intra / (inter + 1e-6) / 0.1

Sharding: feat/label batch-sharded (2048 rows/core); centers statistics
sharded over 512-row slices; the full centers table stays in HBM and is
row-gathered by label via batched indirect DMA. Host all-reduces the
per-core partial sums in float64 and applies the final scalar division.
"""

import numpy as np

B, C, D = 16384, 4096, 128
LAMBDA_C = 1.0
NCORES = 8
BS = B // NCORES          # 2048 feat rows per core
NPT = BS // 128           # 16 feat rows per partition
CS = C // NCORES          # 512 center rows per core (stats slice)
CSPT = CS // 128          # 4 center rows per partition

# schedule knobs (tuned via TimelineSim sweep)
CFG = dict(
    gsplit=(5, 6, 5),      # label-cols (NPT units) per indirect-DMA call
    fsplit=(4, 4, 4, 4),   # label-cols per feat dma_start
    mode="dot",            # cce: gather computes d=c-f via DMA compute op,
                           #      one square-accum per chunk gives intra
                           # dot: separate sum(c^2) and sum(f.c) accums
    dsq_eng=("act", "dve", "act"),         # cce: engine per gather chunk
    dcopy_eng=("dve", "dve", "dve"),       # cce: f->dest copy engine
    csq_eng=("act", "act", "act"),  # dot: per gather chunk: act|dve
    fsq_eng=("dve", "dve", "act", "act"),  # per feat chunk: act|dve
    cs_dma="pool",         # centers-slice load path: sp|pool
    vec_out="split",       # ps_f/ps_c egress: split|copy
    oacc_dma="scatter",    # o_acc egress: sp|pool|scatter (SWDGE prep+trigger)
)

_cached = {}


def _build_nc(cfg=None):
    import concourse.bass as bass
    import concourse.tile as tile
    from concourse import bacc, mybir

    cfg = dict(CFG, **(cfg or {}))
    gsplit = cfg["gsplit"]
    fsplit = cfg["fsplit"]
    assert sum(gsplit) == NPT and sum(fsplit) == NPT

    f32 = mybir.dt.float32
    f32r = mybir.dt.float32r
    i32 = mybir.dt.int32

    nc = bacc.Bacc("TRN2", target_bir_lowering=False, debug=False,
                   num_devices=NCORES)

    feat = nc.dram_tensor("feat", [128, NPT * D], f32, kind="ExternalInput")
    labt = nc.dram_tensor("labt", [128, NPT], i32, kind="ExternalInput")
    centers = nc.dram_tensor("centers", [C, D], f32, kind="ExternalInput")
    # cslice rows flattened to [128, 512] plus a ones column (col 512)
    # that feeds the f32r column-sum matmuls as lhsT
    cslice = nc.dram_tensor("cslice", [128, CSPT * D + 1], f32,
                            kind="ExternalInput")

    i16 = mybir.dt.int16

    NG = len(gsplit)
    NF = len(fsplit)
    # accum cols: fsq[0:NF], csqg[NF:NF+NG], fdotc[NF+NG:NF+2NG], csq last;
    # padded to 64 cols (256B rows) for the scatter-add egress path
    NCOL = NF + 2 * NG + 1
    OW = 16               # scattered cols; DRAM row stays 64 f32 (256B stride)
    assert NCOL <= OW
    o_acc = nc.dram_tensor("o_acc", [128, 64], f32, kind="ExternalOutput")
    o_vec = nc.dram_tensor("o_vec", [1, 1024], f32, kind="ExternalOutput")

    with tile.TileContext(nc) as tc:
        with tc.tile_pool(name="const", bufs=1) as cpool, \
             tc.tile_pool(name="sbuf", bufs=1) as pool, \
             tc.tile_pool(name="scratch", bufs=2) as spool, \
             tc.tile_pool(name="psum", bufs=1, space="PSUM") as psum:



            # centers slice (optionally on the otherwise-idle Pool SWDGE
            # path, freeing an HWDGE slot for the feat loads)
            cs_t = pool.tile([128, CSPT * D + 1], f32, tag="cs_t")
            # f32r-typed DMAs (bit-identical to f32): the BIR verifier
            # requires producers feeding fp32r matmuls to be typed f32r
            if cfg["cs_dma"] == "pool":
                nc.gpsimd.dma_start(out=cs_t[:].bitcast(f32r),
                                    in_=cslice.ap().bitcast(f32r))
            else:
                nc.sync.dma_start(out=cs_t[:].bitcast(f32r),
                                  in_=cslice.ap().bitcast(f32r))
            ones_r = cs_t[:, CSPT * D:CSPT * D + 1].bitcast(f32r)

            o_t = pool.tile([128, OW], f32, tag="o_t")
            if cfg["oacc_dma"] == "scatter":
                nc.vector.memset(o_t[:], 0.0)

            # label indices first so gather desc-gen can start earliest
            lab_t = pool.tile([128, NPT], i32, tag="lab")
            nc.sync.dma_start(out=lab_t[:], in_=labt.ap())
            lab = lab_t[:]

            FW = NPT * D
            f_all = pool.tile([128, FW], f32, tag="f_all")
            fchunks = []
            off = 0
            for w in fsplit:
                fchunks.append((off * D, (off + w) * D))
                off += w
            for a, b in fchunks:
                nc.sync.dma_start(
                    out=f_all[:, a:b].bitcast(f32r),
                    in_=feat.ap()[:, a:b].bitcast(f32r))

            ps_f = psum.tile([1, 512], f32, tag="ps_f")
            ps_c = psum.tile([1, 512], f32, tag="ps_c")

            # centers-slice statistics (independent of feat/gather path)
            cs_scr = spool.tile([128, CSPT * D], f32, tag="cs_scr")
            nc.scalar.activation(out=cs_scr[:], in_=cs_t[:, 0:CSPT * D],
                                 func=mybir.ActivationFunctionType.Square,
                                 accum_out=o_t[:, NCOL - 1:NCOL])
            nc.tensor.matmul(out=ps_c[:], lhsT=ones_r,
                             rhs=cs_t[:, 0:CSPT * D].bitcast(f32r),
                             start=True, stop=True)

            # feat column sums on PE (f32r: 1 cycle/row), fixed 512-wide
            # slices (PSUM bank limit), and squares on ACT per DMA chunk
            for k in range(4):
                sl = f_all[:, k * 512:(k + 1) * 512]
                nc.tensor.matmul(out=ps_f[:], lhsT=ones_r,
                                 rhs=sl.bitcast(f32r),
                                 start=(k == 0), stop=(k == 3))
            for k, (a, b) in enumerate(fchunks):
                wsq = b - a
                sl = f_all[:, a:a + wsq]
                f_scr = spool.tile([128, wsq], f32, tag="f_scr")
                eng = cfg["fsq_eng"][k % len(cfg["fsq_eng"])]
                if eng == "act":
                    nc.scalar.activation(
                        out=f_scr[:], in_=sl,
                        func=mybir.ActivationFunctionType.Square,
                        accum_out=o_t[:, k:k + 1])
                else:
                    with tc.high_priority():
                        nc.vector.scalar_tensor_tensor(
                            out=f_scr[:], in0=sl, scalar=1.0, in1=sl,
                            op0=mybir.AluOpType.mult, op1=mybir.AluOpType.mult,
                            accum_out=o_t[:, k:k + 1])

            # column-sum vectors out (PSUM cannot DMA to DRAM directly,
            # and GPSIMD cannot read PSUM): ps_c copied early on DVE; ps_f
            # folded 512->256 in ONE DVE add issued before the gather
            # accums (it lands in the dead-zone before the first gather sem)
            vec_sb = pool.tile([1, 1024], f32, tag="vec_sb")
            nc.vector.tensor_copy(vec_sb[:, 0:512], ps_c[:])
            nc.vector.tensor_copy(vec_sb[:, 512:1024], ps_f[:])

            # gathered-centers path: batched indirect DMA + per-chunk accums
            cg_all = pool.tile([128, NPT * D], f32, tag="cg_all")
            off = 0
            for g, w in enumerate(gsplit):
                cg = cg_all[:, off * D:(off + w) * D]
                fsl = f_all[:, off * D:(off + w) * D]
                if cfg["mode"] == "cce":
                    # preload dest with f (early), then gather computes
                    # d = c - f in the DMA compute engine; one square-accum
                    # per chunk yields the intra contribution directly
                    ce = cfg["dcopy_eng"][g]
                    if ce == "act":
                        nc.scalar.copy(cg, fsl)
                    elif ce == "pool":
                        nc.gpsimd.tensor_copy(cg, fsl)
                    else:
                        nc.vector.tensor_copy(cg, fsl)
                    nc.gpsimd.indirect_dma_start(
                        out=cg,
                        out_offset=None,
                        in_=centers.ap(),
                        in_offset=bass.IndirectOffsetOnAxis(
                            ap=lab[:, off:off + w], axis=0),
                        compute_op=mybir.AluOpType.subtract,
                    )
                    d_scr = spool.tile([128, w * D], f32, tag="d_scr")
                    if cfg["dsq_eng"][g] == "act":
                        nc.scalar.activation(
                            out=d_scr[:], in_=cg,
                            func=mybir.ActivationFunctionType.Square,
                            accum_out=o_t[:, NF + g:NF + g + 1])
                    else:
                        nc.vector.scalar_tensor_tensor(
                            out=d_scr[:], in0=cg, scalar=1.0, in1=cg,
                            op0=mybir.AluOpType.mult,
                            op1=mybir.AluOpType.mult,
                            accum_out=o_t[:, NF + g:NF + g + 1])
                    off += w
                    continue
                nc.gpsimd.indirect_dma_start(
                    out=cg,
                    out_offset=None,
                    in_=centers.ap(),
                    in_offset=bass.IndirectOffsetOnAxis(
                        ap=lab[:, off:off + w], axis=0),
                )
                if cfg["csq_eng"][g] == "act":
                    c_scr = spool.tile([128, w * D], f32, tag="c_scr")
                    nc.scalar.activation(
                        out=c_scr[:], in_=cg,
                        func=mybir.ActivationFunctionType.Square,
                        accum_out=o_t[:, NF + g:NF + g + 1])
                elif cfg["csq_eng"][g] == "split":
                    # halve across ACT and DVE; DVE half reuses the fdotc
                    # accum layout via an extra column (host sums them all)
                    h = w * D // 2
                    c_scr = spool.tile([128, w * D], f32, tag="c_scr")
                    nc.scalar.activation(
                        out=c_scr[:, 0:h], in_=cg[:, 0:h],
                        func=mybir.ActivationFunctionType.Square,
                        accum_out=o_t[:, NF + g:NF + g + 1])
                    nc.vector.scalar_tensor_tensor(
                        out=c_scr[:, h:w * D], in0=cg[:, h:w * D],
                        scalar=1.0, in1=cg[:, h:w * D],
                        op0=mybir.AluOpType.mult, op1=mybir.AluOpType.mult,
                        accum_out=o_t[:, NCOL:NCOL + 1])
                else:
                    c_scr = spool.tile([128, w * D], f32, tag="c_scr")
                    nc.vector.scalar_tensor_tensor(
                        out=c_scr[:], in0=cg, scalar=1.0, in1=cg,
                        op0=mybir.AluOpType.mult, op1=mybir.AluOpType.mult,
                        accum_out=o_t[:, NF + g:NF + g + 1])
                d_scr = spool.tile([128, w * D], f32, tag="d_scr")
                nc.vector.scalar_tensor_tensor(
                    out=d_scr[:], in0=fsl, scalar=1.0, in1=cg,
                    op0=mybir.AluOpType.mult, op1=mybir.AluOpType.mult,
                    accum_out=o_t[:, NF + NG + g:NF + NG + g + 1])
                off += w

            if cfg["oacc_dma"] == "scatter":
                # SWDGE prep + trigger egress: the trigger path skips the
                # HWDGE/DGE setup (~1.3us) off the critical tail. The zero
                # fill rides the Pool SWDGE queue so its desc-gen (and hence
                # its transfer) naturally falls behind the gather stream;
                # the iota and descriptor prep follow it (WAW order: zero
                # write precedes the scatter-add), all before the vec copies.
                zs = cpool.tile([128, OW], f32)
                nc.vector.memset(zs[:], 0.0)
                # de-prioritize so the scheduler keeps the zero fill, iota
                # and prep behind the gather stream
                p0 = tc.cur_priority
                tc.cur_priority = p0 + 100000
                nc.sync.dma_start(out=o_acc.ap()[:, 0:OW], in_=zs[:])
                sidx = cpool.tile([128, 8], i16)
                nc.vector.memset(sidx[:], 0)
                nc.gpsimd.iota(sidx[0:16, :], pattern=[[16, 8]], base=0,
                               channel_multiplier=1)
                oacc_sem = nc.alloc_semaphore("oacc_dma")
                nc.gpsimd.dma_scatter_add(
                    o_acc.ap()[:, 0:OW], o_t[:].unsqueeze(1), sidx[:],
                    128, 128, OW, elem_step=64,
                    prepare_only=True, sem=oacc_sem)
                tc.cur_priority = p0

            nc.sync.dma_start(out=o_vec.ap(), in_=vec_sb[:])

            if cfg["oacc_dma"] == "scatter":
                nc.gpsimd.trigger_dma(count=None)
            elif cfg["oacc_dma"] == "pool":
                nc.gpsimd.dma_start(out=o_acc.ap(), in_=o_t[:])
            else:
                nc.sync.dma_start(out=o_acc.ap(), in_=o_t[:])

    if cfg["oacc_dma"] == "scatter":
        # The prep's explicit completion sem occupies the descriptor's single
        # sem_num slot, so the auto-assigned DMASW lane sem never increments;
        # the exit drain still waits on it. Repoint those orphaned waits at
        # the real completion sem (same +16 contract).
        fn = nc.m.functions[0]
        updated = set()
        oacc_id = None
        for blk in fn.blocks:
            for inst in blk.instructions:
                si = inst.sync_info
                if si is None:
                    continue
                for u in si.on_update:
                    updated.add(u.id)
                    if u.ant_name == "oacc_dma":
                        oacc_id = u.id
        assert oacc_id is not None
        for blk in fn.blocks:
            for inst in blk.instructions:
                si = inst.sync_info
                if si is None:
                    continue
                for w in si.on_wait:
                    if (w.ant_name and w.ant_name.startswith("DMASW")
                            and w.id not in updated):
                        w.id = oacc_id
                        w.ant_name = "oacc_dma"

    nc.compile()
    return nc


def _get_nc():
    if "nc" not in _cached:
        _cached["nc"] = _build_nc()
    return _cached["nc"]


def _make_in_maps(feat, label, centers):
    feat = np.ascontiguousarray(np.asarray(feat, dtype=np.float32))
    centers = np.ascontiguousarray(np.asarray(centers, dtype=np.float32))
    lab = np.asarray(label).astype(np.int32)
    in_maps = []
    for k in range(NCORES):
        fs = feat[k * BS:(k + 1) * BS].reshape(128, NPT * D)
        ls = lab[k * BS:(k + 1) * BS].reshape(128, NPT)
        cs = np.concatenate(
            [centers[k * CS:(k + 1) * CS].reshape(128, CSPT * D),
             np.ones((128, 1), dtype=np.float32)], axis=1)
        in_maps.append({
            "feat": np.ascontiguousarray(fs),
            "labt": np.ascontiguousarray(ls),
            "centers": centers,
            "cslice": np.ascontiguousarray(cs),
        })
    return in_maps


def _combine(results):
    NF = len(CFG["fsplit"])
    NG = len(CFG["gsplit"])
    sum_fsq = 0.0
    sum_csqg = 0.0
    sum_fdotc = 0.0
    sum_csq = 0.0
    F = np.zeros(D, dtype=np.float64)
    Cv = np.zeros(D, dtype=np.float64)
    for r in results:
        acc = r["o_acc"].astype(np.float64)
        sum_fsq += acc[:, 0:NF].sum()
        sum_csqg += acc[:, NF:NF + NG].sum()
        if "split" in CFG["csq_eng"]:
            sum_csqg += acc[:, NF + 2 * NG + 1].sum()
        sum_fdotc += acc[:, NF + NG:NF + 2 * NG].sum()
        sum_csq += acc[:, NF + 2 * NG].sum()
        v = r["o_vec"][0].astype(np.float64)
        Cv += v[:512].reshape(4, 128).sum(axis=0)
        F += v[512:].reshape(4, 128).sum(axis=0)
    if CFG["mode"] == "cce":
        intra = sum_csqg          # those columns hold sum (c - f)^2
    else:
        intra = sum_fsq + sum_csqg - 2.0 * sum_fdotc
    total = C * sum_fsq + B * sum_csq - 2.0 * float(F @ Cv)
    inter = total - intra
    loss = (LAMBDA_C / 2.0 / B) * intra / (inter + 1e-6) / 0.1
    return np.float32(loss)


def kernel(feat, label, centers):
    from concourse.bass_utils import run_bass_kernel_spmd

    nc = _get_nc()
    in_maps = _make_in_maps(feat, label, centers)
    res = run_bass_kernel_spmd(nc, in_maps, list(range(NCORES)))
    return _combine(res.results)
